# revision 21
# baseline (speedup 1.0000x reference)
"""Trainium2 Bass kernel for nn_AxialBlock (3-axis axial attention sum).

Problem (hardcoded): x (B=4, C=512, T=16, H=32, W=32) fp32, three axial
MHA blocks (attend along W, H, T; n_head=8, d=64) each with their own
QKVO projections; outputs summed. Output (B, C, T, H, W) fp32.

Sharding: 8 cores = (batch b in 0..3) x (H-half j in 0..1). Every pass is
computed fully locally (no collectives):
  - w-pass / t-pass: tokens (t, h in owned half, w), fully local.
  - h-pass: attention along H needs all H, so the full batch sample is
    recomputed on both cores of a pair; each core keeps only its owned
    H-half of the output. (For odd cores the H axis is rotated host-side
    so the owned half is always h-positions 0..15 — attention along H is
    permutation-equivariant, so this is exact.)

On-device layout trick: x is channels-first, i.e. already "x^T" (C on
partitions) which is what the PE wants for the QKV projections. The host
pre-permutes x into three token orders (w-fastest / t-fastest / h-fastest)
so that each axial attention acts on 32 consecutive tokens ("rows").

Per 512-token tile (16 rows x 32 tokens):
  q^T, k^T (feat-partition) and v (token-partition) projections in bf16,
  per-(row, head) 32x32 attention scores via PE array tiling (K=64 mode),
  softmax along free dim (exp on ScalarE, per-block reduce on VectorE),
  A -> A^T via the DVE 32x32 stream transpose, o^T = V^T A^T via PE
  (32x64 tiling, writes o^T feat-partition directly), out-projection,
  and accumulation of the three passes through DRAM read-modify-write.

t-axis has seq len 16: two t-fibers are packed into one 32-token row and
a 0/1 block mask zeroes cross-fiber attention after exp.
"""

import contextlib

import ml_dtypes
import numpy as np

import concourse.bass as bass
import concourse.tile as tile
from concourse import bacc, mybir
from concourse.bass_utils import run_bass_kernel_spmd

BF16 = mybir.dt.bfloat16
FP32 = mybir.dt.float32
BF16_NP = np.dtype(ml_dtypes.bfloat16)

B, C, T, H, W = 4, 512, 16, 32, 32
NH, D = 8, 64
HL = H // 2              # per-core H slice
N_CORES = 8
TOK_LOCAL = T * HL * W   # 8192 tokens owned per core
TOK_FULL = T * H * W     # 16384 tokens in a batch sample
TILE = 512               # tokens per on-chip tile
NCH = C // 128           # 4 partition chunks of the feature dim

# dev knob: cap tiles per pass (None = full problem). Truncated builds are
# only for fast AP/scheduling smoke tests - output is wrong when set.
NTILES_CAP = None


def _build_pass(tc, pools, axis, x_ap, w_aps, y_ap, bias_aps, tmask_sb,
                pmask_aps, abd_tiles):
    """Emit one axial-attention pass.

    axis: 'w' | 't' | 'h'.  x_ap: (512, ntok) bf16 DRAM, token order chosen
    so each 32-token group is one attention row.  y_ap: (512, 8192) fp32
    DRAM output accumulator (natural (t, h_local, w) token order).
    """
    nc = tc.nc
    wq_sb, wk_sb, wv_sb, wo_sb = w_aps
    ntok = TOK_FULL if axis == "h" else TOK_LOCAL
    ntiles = ntok // TILE
    if NTILES_CAP is not None:
        ntiles = min(ntiles, NTILES_CAP)

    (xt_pool, qk_pool, v_pool, a_pool, sm_pool,
     ot_pool, y_pool, ps_pool, sps_pool) = pools

    # y viewed (c, t, hl, w) for the strided rmw accumulation
    y4d = y_ap.rearrange("c (t h w) -> c t h w", t=T, h=HL, w=W)

    for it in range(ntiles):
        # ---- load x^T tile: (128, NCH, TILE) bf16, free = (chunk, token)
        xt = xt_pool.tile([128, NCH, TILE], BF16)
        for kc in range(NCH):
            nc.sync.dma_start(
                xt[:, kc, :], x_ap[128 * kc:128 * (kc + 1), it * TILE:(it + 1) * TILE]
            )

        # ---- q^T, k^T projections: feat-partition (128, NCH, TILE) bf16
        q_sb = qk_pool.tile([128, NCH, TILE], BF16, tag="q")
        k_sb = qk_pool.tile([128, NCH, TILE], BF16, tag="k")
        for w_sb, dst, ev in ((wq_sb, q_sb, 0), (wk_sb, k_sb, 1)):
            for mc in range(NCH):
                ps = ps_pool.tile([128, TILE], FP32, tag="ps", bufs=2)
                for kc in range(NCH):
                    nc.tensor.matmul(
                        ps[:],
                        lhsT=w_sb[:, kc, 128 * mc:128 * (mc + 1)],
                        rhs=xt[:, kc, :],
                        start=(kc == 0), stop=(kc == NCH - 1),
                    )
                if ev == 0:
                    nc.scalar.copy(dst[:, mc, :], ps[:])
                else:
                    nc.vector.tensor_copy(dst[:, mc, :], ps[:])

        # ---- v projection, token-partition: (128, NCH, C) bf16,
        #      free = (token block ts, feature)
        v_sb = v_pool.tile([128, NCH, C], BF16)
        for ts in range(NCH):
            ps = ps_pool.tile([128, TILE], FP32, tag="ps", bufs=2)
            for kc in range(NCH):
                nc.tensor.matmul(
                    ps[:],
                    lhsT=xt[:, kc, 128 * ts:128 * (ts + 1)],
                    rhs=wv_sb[:, kc, :],
                    start=(kc == 0), stop=(kc == NCH - 1),
                )
            if ts % 2 == 0:
                nc.scalar.copy(v_sb[:, ts, :], ps[:])
            else:
                nc.vector.tensor_copy(v_sb[:, ts, :], ps[:])

        # ---- parity-masked k (k_z): zero the other head's 64 d-rows so the
        # S matmul can contract over all 128 partitions (only legal PE tile
        # positions are row 0 / col 0 — see module docstring). Built on the
        # otherwise idle GpSimd engine.
        kz_sb = v_pool.tile([128, 2, NCH, TILE], BF16, tag="kz")
        for par in range(2):
            for c in range(NCH):
                nc.gpsimd.tensor_scalar_mul(
                    kz_sb[:, par, c, :], k_sb[:, c, :], pmask_aps[par]
                )

        # ---- attention: 16 rows x 8 heads of 32x32 blocks
        # o^T accumulator psum per feature chunk: (128, TILE)
        ot_ps = [
            ps_pool.tile([128, TILE], FP32, name=f"ot{c}", tag=f"ot{c}", bufs=1)
            for c in range(NCH)
        ]
        for g in range(4):           # row group: 4 rows = 128 tokens
            # scores: S[(4 rows)x32q, (chunk c, head p)x32k] fp32
            sps = sps_pool.tile([128, NH * 32], FP32)
            for c in range(NCH):
                for p in range(2):
                    for j in range(4):
                        col = (g * 4 + j) * 32
                        nc.tensor.matmul(
                            sps[32 * j:32 * (j + 1),
                                (2 * c + p) * 32:(2 * c + p + 1) * 32],
                            lhsT=q_sb[:, c, col:col + 32],
                            rhs=kz_sb[:, p, c, col:col + 32],
                            tile_position=(0, 32 * j),
                        )
            # softmax along k (free dim within each 32-block)
            a_sb = a_pool.tile([128, NH * 32], BF16, tag="a")
            nc.scalar.activation(a_sb[:], sps[:], mybir.ActivationFunctionType.Exp)
            a3 = a_sb[:].rearrange("p (n k) -> p n k", n=NH)
            if axis == "t":
                nc.vector.tensor_tensor(
                    a3, a3,
                    tmask_sb[:].unsqueeze(1).broadcast_to((128, NH, 32)),
                    mybir.AluOpType.mult,
                )
            sums = sm_pool.tile([128, NH], FP32, tag="sums")
            nc.vector.tensor_reduce(
                sums[:], a3, axis=mybir.AxisListType.X, op=mybir.AluOpType.add
            )
            recip = sm_pool.tile([128, NH], FP32, tag="recip")
            nc.vector.reciprocal(recip[:], sums[:])
            nc.vector.tensor_tensor(
                a3, a3,
                recip[:].unsqueeze(2).broadcast_to((128, NH, 32)),
                mybir.AluOpType.mult,
            )
            # A -> block-diagonal A^T (a_bd): per row j, transpose all 8
            # heads' (32q x 32k) blocks into kpos-partition rows. a_bd
            # columns are (row j)*256 + head*32 + q, so each call writes a
            # contiguous (32, 256) slab on partitions [32j, 32j+32); all
            # other partitions in that slab stay zero from the one-time
            # memset (persistent double buffers).
            abd = abd_tiles[g % 2]
            for j in range(4):
                nc.vector.transpose(
                    abd[32 * j:32 * (j + 1), 256 * j:256 * (j + 1)],
                    a_sb[32 * j:32 * (j + 1), :],
                )

            # o^T chunk c = V^T A_bd per head: one matmul per (head, rowgroup)
            abd4 = abd[:].rearrange("p (j n q) -> p j n q", j=4, n=NH)
            for c in range(NCH):
                for p in range(2):
                    nc.tensor.matmul(
                        ot_ps[c][64 * p:64 * (p + 1), g * 128:(g + 1) * 128],
                        lhsT=v_sb[:, g, (2 * c + p) * 64:(2 * c + p + 1) * 64],
                        rhs=abd4[:, :, 2 * c + p, :],
                        tile_position=(0, 64 * p),
                    )

        # ---- evacuate o^T to bf16 SBUF
        ot_sb = ot_pool.tile([128, NCH, TILE], BF16)
        for c in range(NCH):
            if c % 2 == 0:
                nc.scalar.copy(ot_sb[:, c, :], ot_ps[c][:])
            else:
                nc.vector.tensor_copy(ot_sb[:, c, :], ot_ps[c][:])

        # ---- out-projection + accumulate into y
        for mc in range(NCH):
            yps = ps_pool.tile([128, TILE], FP32, tag="yps", bufs=1)
            for kc in range(NCH):
                nc.tensor.matmul(
                    yps[:],
                    lhsT=wo_sb[:, kc, 128 * mc:128 * (mc + 1)],
                    rhs=ot_sb[:, kc, :],
                    start=(kc == 0), stop=(kc == NCH - 1),
                )
            cs = slice(128 * mc, 128 * (mc + 1))
            if axis == "w":
                # first pass: plain write, fold the (summed) output bias in
                y_sb = y_pool.tile([128, TILE], FP32, tag="yw")
                nc.scalar.activation(
                    y_sb[:], yps[:], mybir.ActivationFunctionType.Identity,
                    bias=bias_aps[mc],
                )
                nc.sync.dma_start(y_ap[cs, it * TILE:(it + 1) * TILE], y_sb[:])
            elif axis == "t":
                # tile it covers h-row `it`; psum tokens are (w 32, t 16)
                # t-fastest, DRAM side stays natural (t-major, w contiguous)
                y_slice = y4d[cs, :, it, :]                       # (128, t16, w32)
                yprev = y_pool.tile([128, T, W], FP32, tag="yt")
                nc.sync.dma_start(yprev[:], y_slice)
                ynew = y_pool.tile([128, T, W], FP32, tag="yt2")
                yp3 = yps[:].rearrange("p (w t) -> p w t", w=W).transpose([0, 2, 1])
                nc.vector.tensor_tensor(
                    ynew[:], yprev[:], yp3, mybir.AluOpType.add
                )
                nc.sync.dma_start(y_slice, ynew[:])
            else:
                # h-pass: tile it covers t = it//2, w-half = it%2, tokens
                # (tw 16, h 32) h-fastest; owned h is always positions 0..15
                t_idx, w_half = it // 2, it % 2
                ws = slice(16 * w_half, 16 * (w_half + 1))
                y_slice = y4d[cs, t_idx, :, ws]                   # (128, hl16, w16)
                yprev = y_pool.tile([128, HL, 16], FP32, tag="yh")
                nc.sync.dma_start(yprev[:], y_slice)
                ynew = y_pool.tile([128, HL, 16], FP32, tag="yh2")
                yp3 = (yps[:].rearrange("p (w h) -> p w h", w=16)[:, :, 0:HL]
                       .transpose([0, 2, 1]))
                nc.vector.tensor_tensor(
                    ynew[:], yprev[:], yp3, mybir.AluOpType.add
                )
                nc.sync.dma_start(y_slice, ynew[:])


def build_program():
    """Build + compile the SPMD bass program (same program on all 8 cores)."""
    nc = bacc.Bacc(
        "TRN2", target_bir_lowering=False, debug=False,
        enable_asserts=False, num_devices=N_CORES,
    )

    def din(name, shape, dt=BF16):
        return nc.dram_tensor(name, shape, dt, kind="ExternalInput").ap()

    x_w = din("x_w", (C, TOK_LOCAL))
    x_t = din("x_t", (C, TOK_LOCAL))
    x_h = din("x_h", (C, TOK_FULL))
    w_in = {}
    for ax in ("w", "t", "h"):
        for nm in ("wq", "wk", "wv", "wo"):
            w_in[f"{nm}_{ax}"] = din(f"{nm}_{ax}", (C, C))
    bias_in = din("bias", (C, 1), FP32)
    tmask_in = din("tmask", (128, 32))
    pmask_in = din("pmask", (128, 2), FP32)
    y_ap = nc.dram_tensor("y", (C, TOK_LOCAL), FP32, kind="ExternalOutput").ap()

    with tile.TileContext(nc) as tc:
        with contextlib.ExitStack() as ctx:
            xt_pool = ctx.enter_context(tc.tile_pool(name="xt", bufs=3))
            w_pool = ctx.enter_context(tc.tile_pool(name="wts", bufs=2))
            qk_pool = ctx.enter_context(tc.tile_pool(name="qk", bufs=2))
            v_pool = ctx.enter_context(tc.tile_pool(name="v", bufs=2))
            a_pool = ctx.enter_context(tc.tile_pool(name="a", bufs=3))
            sm_pool = ctx.enter_context(tc.tile_pool(name="sm", bufs=3))
            ot_pool = ctx.enter_context(tc.tile_pool(name="ot", bufs=2))
            y_pool = ctx.enter_context(tc.tile_pool(name="y", bufs=3))
            ps_pool = ctx.enter_context(tc.tile_pool(name="ps", bufs=2, space="PSUM"))
            sps_pool = ctx.enter_context(tc.tile_pool(name="sps", bufs=1, space="PSUM"))
            const_pool = ctx.enter_context(tc.tile_pool(name="const", bufs=1))

            # constants
            tmask_sb = const_pool.tile([128, 32], BF16)
            nc.sync.dma_start(tmask_sb[:], tmask_in[:])
            pmask_sb = const_pool.tile([128, 2], FP32)
            nc.sync.dma_start(pmask_sb[:], pmask_in[:])
            pmask_aps = [pmask_sb[:, par:par + 1] for par in range(2)]
            bias_sb = const_pool.tile([128, NCH], FP32)
            for mc in range(NCH):
                nc.sync.dma_start(
                    bias_sb[:, mc:mc + 1], bias_in[128 * mc:128 * (mc + 1), :]
                )
            bias_aps = [bias_sb[:, mc:mc + 1] for mc in range(NCH)]

            # persistent block-diagonal A^T double buffers, zeroed once
            abd0 = const_pool.tile([128, 4 * NH * 32], BF16)
            abd1 = const_pool.tile([128, 4 * NH * 32], BF16)
            nc.gpsimd.memset(abd0[:], 0.0)
            nc.gpsimd.memset(abd1[:], 0.0)
            abd_tiles = [abd0, abd1]

            pools = (xt_pool, qk_pool, v_pool, a_pool, sm_pool,
                     ot_pool, y_pool, ps_pool, sps_pool)

            for ax, x_ap in (("w", x_w), ("t", x_t), ("h", x_h)):
                w_aps = []
                for nm in ("wq", "wk", "wv", "wo"):
                    wt = w_pool.tile([128, NCH, C], BF16, tag=nm)
                    for kc in range(NCH):
                        nc.sync.dma_start(
                            wt[:, kc, :],
                            w_in[f"{nm}_{ax}"][128 * kc:128 * (kc + 1), :],
                        )
                    w_aps.append(wt)
                _build_pass(tc, pools, ax, x_ap, w_aps, y_ap, bias_aps, tmask_sb,
                            pmask_aps, abd_tiles)

    nc.compile()
    return nc


_PROGRAM = None


def _get_program():
    global _PROGRAM
    if _PROGRAM is None:
        _PROGRAM = build_program()
    return _PROGRAM


def make_in_maps(inputs):
    """Host-side shard + layout prep: per-core input dicts."""
    x = np.asarray(inputs["x"], np.float32)          # (B, C, T, H, W)
    scale = 1.0 / np.sqrt(D)

    weights = {}
    for ax in ("w", "h", "t"):
        for nm in ("wq", "wk", "wv", "wo"):
            wm = np.asarray(inputs[f"{nm}_{ax}"], np.float32)
            if nm == "wq":
                wm = wm * scale
            # lhsT layout: (C_in, C_out) = W.T
            weights[f"{nm}_{ax}"] = np.ascontiguousarray(wm.T).astype(BF16_NP)
    bias = (np.asarray(inputs["bo_w"], np.float32)
            + np.asarray(inputs["bo_h"], np.float32)
            + np.asarray(inputs["bo_t"], np.float32)).reshape(C, 1)

    # t-pass cross-fiber 0/1 mask: partitions = 4 row-blocks x 32 qpos,
    # free = 32 kpos; two 16-long t-fibers per 32-token row.
    p = np.arange(128) % 32
    k = np.arange(32)
    tmask = ((p[:, None] // 16) == (k[None, :] // 16)).astype(BF16_NP)
    # head-parity masks for k_z: col par = 1 on partitions [64*par, 64*par+64)
    pmask = np.stack([(np.arange(128) // 64) == par for par in range(2)],
                     axis=1).astype(np.float32)

    in_maps = []
    for core in range(N_CORES):
        b, j = divmod(core, 2)
        xb = x[b]                                    # (C, T, H, W)
        xw = xb[:, :, 16 * j:16 * (j + 1), :]        # (C, T, HL, W) w-fastest
        xt = np.transpose(xw, (0, 2, 3, 1))          # (C, HL, W, T) t-fastest
        xh = np.transpose(xb, (0, 1, 3, 2))          # (C, T, W, H) h-fastest
        if j == 1:
            # rotate H so the owned half is always h-positions 0..15
            xh = np.concatenate([xh[..., 16:], xh[..., :16]], axis=-1)
        m = {
            "x_w": np.ascontiguousarray(xw).reshape(C, TOK_LOCAL).astype(BF16_NP),
            "x_t": np.ascontiguousarray(xt).reshape(C, TOK_LOCAL).astype(BF16_NP),
            "x_h": np.ascontiguousarray(xh).reshape(C, TOK_FULL).astype(BF16_NP),
            "bias": bias, "tmask": tmask, "pmask": pmask,
        }
        m.update(weights)
        in_maps.append(m)
    return in_maps


def assemble_output(results):
    """Gather per-core y (C, 8192) into (B, C, T, H, W) fp32."""
    out = np.empty((B, C, T, H, W), np.float32)
    for core in range(N_CORES):
        b, j = divmod(core, 2)
        y = np.asarray(results[core]["y"]).reshape(C, T, HL, W)
        out[b, :, :, 16 * j:16 * (j + 1), :] = y
    return out


def kernel(**inputs) -> np.ndarray:
    nc = _get_program()
    in_maps = make_in_maps(inputs)
    res = run_bass_kernel_spmd(nc, in_maps, core_ids=list(range(N_CORES)))
    return assemble_output(res.results)
